# revision 7
# baseline (speedup 1.0000x reference)
"""ImprovedGRUCell Trainium2 kernel (8-core data-parallel over batch).

v4 layout strategy — fully transposed (feature-major) compute:
  - batch sharded 8 ways (8192 rows/core); 256x256 weights replicated.
  - Host pre-transposes x/h to [256, B] bf16 per core, so activations load
    feature-major directly: SBUF tiles [128 part = feature, free = batch].
    No on-device transposes.
  - All GEMMs in transposed orientation S^T[h, b] = W @ x^T: weight tile
    stationary (lhsT = W.T block [128k, 128h]), activation moving (N=512).
    24 matmuls of [128,128]x[128,512] per 512-column super-tile.
  - softmax over hidden (partition dim): va scale + exp are per-partition
    ACT ops.  Denominator: ones_col.T @ E -> [1,512] psum row; ACT-copies
    to SBUF; rank-1 matmul (ones_row x row) re-broadcasts to [128,512]
    psum; reciprocal_approx_fast gives rb.  Normalization folded into h
    before att = E * (h * rb), which feeds U_h directly.
  - sigmoid avoided: z = 0.5*tanh(s/2)+0.5; blend h_t = h + u*(htl-h)
    with u = 0.5*tz+0.5 (tensor_scalar hits DVE 2x perf mode).
  - 3-deep software pipeline; per-engine FIFO order matched to operand
    readiness: PE = cand(it-2), bcast(it-1), attn(it), z(it), denom(it);
    ACT = htl, A, E, dr, tz; DVE = u, m, recip, h*rb, att.  x/h loads
    prefetch one iteration ahead; all 6 weight matrices ship as one DMA;
    blend-add + store run on gpsimd to keep DVE/SP queues clear.
  - Output written bf16 transposed [256, B]; host transposes/casts to f32.
"""

import sys

sys.path.insert(0, "/opt/trn_rl_repo")

import ml_dtypes
import numpy as np

import concourse.bass as bass
import concourse.mybir as mybir
from concourse import bacc, tile
from concourse.bass_utils import run_bass_kernel_spmd

B_TOTAL = 65536
N_CORES = 8
B_CORE = B_TOTAL // N_CORES  # 8192
D = 256
ST = 512  # batch columns per super-tile
N_ST = B_CORE // ST  # 16
NW = 6  # weight matrices

F32 = mybir.dt.float32
BF16 = mybir.dt.bfloat16
AF = mybir.ActivationFunctionType
ALU = mybir.AluOpType

_CACHE = {}

# order inside the packed weight tensor
WNAMES = ("wzt", "uzt", "wat", "uat", "wht", "uht")
WIDX = {n: i for i, n in enumerate(WNAMES)}


def build_nc(use_bias=False):
    nc = bacc.Bacc(
        "TRN2",
        target_bir_lowering=False,
        debug=False,
        enable_asserts=False,
        num_devices=N_CORES,
    )

    x_d = nc.dram_tensor("xT", [D, B_CORE], BF16, kind="ExternalInput")
    h_d = nc.dram_tensor("hT", [D, B_CORE], BF16, kind="ExternalInput")
    w_d = nc.dram_tensor("wall", [D, NW * D], BF16, kind="ExternalInput")
    va_d = nc.dram_tensor("va", [D], F32, kind="ExternalInput")
    bz_d = nc.dram_tensor("bzh", [128, 2], F32, kind="ExternalInput")
    bh_d = nc.dram_tensor("bh", [128, 2], F32, kind="ExternalInput")
    out_d = nc.dram_tensor("out", [D, B_CORE], BF16, kind="ExternalOutput")

    with tile.TileContext(nc) as tc:
        with (
            tc.tile_pool(name="wpool", bufs=1) as wp,
            tc.tile_pool(name="io", bufs=4) as io,
            tc.tile_pool(name="wk", bufs=2) as wk,
            tc.tile_pool(name="psz", bufs=1, space="PSUM") as psZ,
            tc.tile_pool(name="psa", bufs=1, space="PSUM") as psA,
            tc.tile_pool(name="psc", bufs=1, space="PSUM") as psC,
            tc.tile_pool(name="psd", bufs=1, space="PSUM") as psD,
            tc.tile_pool(name="psr", bufs=1, space="PSUM") as psR,
        ):
            # ---- persistent weights (single DMA) ------------------------
            # wt[p, kb*(NW*D) + n*D + h] = Wn.T[kb*128 + p, h]
            wt = wp.tile([128, 2 * NW * D], BF16, tag="wt")
            nc.sync.dma_start(
                out=wt.rearrange("p (kb c) -> p kb c", kb=2),
                in_=w_d.ap().rearrange("(kb p) c -> p kb c", p=128),
            )

            def wsl(n, kb, hb):
                e = kb * (NW * D) + WIDX[n] * D + hb * 128
                return wt[:, e : e + 128]

            va_sb = wp.tile([128, 2], F32, tag="va")
            nc.sync.dma_start(
                out=va_sb[:], in_=va_d.ap().rearrange("(t p) -> p t", p=128)
            )
            ones_c = wp.tile([128, 1], BF16, tag="ones_c")
            nc.vector.memset(ones_c[:], 1.0)
            ones_r = wp.tile([1, 128], BF16, tag="ones_r")
            nc.vector.memset(ones_r[:], 1.0)
            if use_bias:
                bz_sb = wp.tile([128, 2], F32, tag="bz")
                nc.sync.dma_start(out=bz_sb[:], in_=bz_d.ap())
                bh_sb = wp.tile([128, 2], F32, tag="bh")
                nc.sync.dma_start(out=bh_sb[:], in_=bh_d.ap())

            # cross-iteration tile refs for the software pipeline
            xts, hts, tzs, Es, drs, atts, htls, dds = (
                {}, {}, {}, {}, {}, {}, {}, {},
            )

            def loads(st):
                b0 = st * ST
                xt = io.tile([128, 2 * ST], BF16, tag="xt", bufs=5)
                nc.sync.dma_start(
                    out=xt.rearrange("p (kb b) -> p kb b", kb=2),
                    in_=x_d.ap()[:, b0 : b0 + ST].rearrange(
                        "(kb p) b -> p kb b", p=128
                    ),
                )
                ht = io.tile([128, 2 * ST], BF16, tag="ht", bufs=6)
                nc.sync.dma_start(
                    out=ht.rearrange("p (kb b) -> p kb b", kb=2),
                    in_=h_d.ap()[:, b0 : b0 + ST].rearrange(
                        "(kb p) b -> p kb b", p=128
                    ),
                )
                xts[st], hts[st] = xt, ht

            def mm_pair(ps, wx, wh, rx, rh):
                """8 matmuls: ps[:, hb*ST:+ST] += Wx@rx + Wh@rh (2 k-blocks)."""
                for hb in range(2):
                    o = ps[:, hb * ST : (hb + 1) * ST]
                    for kb in range(2):
                        nc.tensor.matmul(
                            o,
                            wsl(wx, kb, hb),
                            rx[:, kb * ST : (kb + 1) * ST],
                            start=(kb == 0),
                            stop=False,
                        )
                    for kb in range(2):
                        nc.tensor.matmul(
                            o,
                            wsl(wh, kb, hb),
                            rh[:, kb * ST : (kb + 1) * ST],
                            start=False,
                            stop=(kb == 1),
                        )

            for it in range(N_ST + 3):
                # ==== input prefetch (one iteration ahead) ==============
                if it == 0:
                    loads(0)
                if it + 1 < N_ST:
                    loads(it + 1)

                # ==== stage D (it-3): blend + store =====================
                bt = it - 3
                if bt >= 0:
                    htb = hts.pop(bt)
                    tzb, ddb = tzs.pop(bt), dds.pop(bt)
                    uu = wk.tile([128, 2 * ST], BF16, tag="uu")
                    nc.vector.tensor_scalar(
                        uu[:], tzb[:], 0.5, 0.5, op0=ALU.mult, op1=ALU.add
                    )
                    mm_ = wk.tile([128, 2 * ST], BF16, tag="mm_")
                    nc.vector.tensor_mul(mm_[:], uu[:], ddb[:])
                    ot = io.tile([128, 2 * ST], BF16, tag="ot", bufs=3)
                    nc.gpsimd.tensor_add(ot[:], mm_[:], htb[:])
                    nc.gpsimd.dma_start(
                        out=out_d.ap()[:, bt * ST : (bt + 1) * ST].rearrange(
                            "(hb p) b -> p hb b", p=128
                        ),
                        in_=ot.rearrange("p (hb b) -> p hb b", hb=2),
                    )

                # ==== stage C (it-2): candidate branch + tanh + sub =====
                jt = it - 2
                if 0 <= jt < N_ST:
                    xtj = xts.pop(jt)
                    attj = atts.pop(jt)
                    pc = psC.tile([128, 2 * ST], F32, tag="pc")
                    mm_pair(pc, "wht", "uht", xtj, attj)
                    htl = wk.tile([128, 2 * ST], BF16, tag="htl", bufs=3)
                    if use_bias:
                        for hb in range(2):
                            sl = slice(hb * ST, (hb + 1) * ST)
                            nc.scalar.activation(
                                htl[:, sl], pc[:, sl], AF.Tanh,
                                bias=bh_sb[:, hb : hb + 1],
                            )
                    else:
                        nc.scalar.activation(htl[:], pc[:], AF.Tanh)
                    htls[jt] = htl
                    dd = wk.tile([128, 2 * ST], BF16, tag="dd", bufs=3)
                    nc.gpsimd.tensor_sub(dd[:], htl[:], hts[jt][:])
                    dds[jt] = dd

                # ==== stage B (it-1): denom bcast + normalize ===========
                kt = it - 1
                if 0 <= kt < N_ST:
                    rbp = psR.tile([128, ST], F32, tag="rbp")
                    nc.tensor.matmul(
                        rbp[:], ones_r[:], drs.pop(kt)[:], start=True, stop=True
                    )
                    rb = wk.tile([128, ST], F32, tag="rb")
                    nc.vector.reciprocal_approx_fast(out=rb[:], in_=rbp[:])
                    hr = wk.tile([128, 2 * ST], BF16, tag="hr")
                    for kb in range(2):
                        sl = slice(kb * ST, (kb + 1) * ST)
                        nc.vector.tensor_mul(hr[:, sl], hts[kt][:, sl], rb[:])
                    att = wk.tile([128, 2 * ST], BF16, tag="att", bufs=3)
                    nc.vector.tensor_mul(att[:], Es.pop(kt)[:], hr[:])
                    atts[kt] = att

                if it >= N_ST:
                    continue

                # ==== stage A: attention + z branches for tile `it` =====
                xt, ht = xts[it], hts[it]
                pa = psA.tile([128, 2 * ST], F32, tag="pa")
                mm_pair(pa, "wat", "uat", xt, ht)
                A = wk.tile([128, 2 * ST], BF16, tag="A")
                nc.scalar.activation(A[:], pa[:], AF.Tanh)
                E = wk.tile([128, 2 * ST], BF16, tag="E", bufs=3)
                for hb in range(2):
                    sl = slice(hb * ST, (hb + 1) * ST)
                    nc.scalar.activation(
                        E[:, sl], A[:, sl], AF.Exp, scale=va_sb[:, hb : hb + 1]
                    )
                Es[it] = E

                pz = psZ.tile([128, 2 * ST], F32, tag="pz")
                mm_pair(pz, "wzt", "uzt", xt, ht)

                # softmax denominator row [1, ST] (sum over all 256 h)
                pd = psD.tile([1, ST], F32, tag="pd")
                for hb in range(2):
                    nc.tensor.matmul(
                        pd[:],
                        ones_c[:],
                        E[:, hb * ST : (hb + 1) * ST],
                        start=(hb == 0),
                        stop=(hb == 1),
                    )
                dr = wk.tile([1, ST], BF16, tag="dr", bufs=2)
                nc.scalar.activation(dr[:], pd[:], AF.Copy)
                drs[it] = dr

                tz = wk.tile([128, 2 * ST], BF16, tag="tz", bufs=4)
                if use_bias:
                    for hb in range(2):
                        sl = slice(hb * ST, (hb + 1) * ST)
                        nc.scalar.activation(
                            tz[:, sl], pz[:, sl], AF.Tanh,
                            bias=bz_sb[:, hb : hb + 1], scale=0.5,
                        )
                else:
                    nc.scalar.activation(tz[:], pz[:], AF.Tanh, scale=0.5)
                tzs[it] = tz

    nc.compile()
    return nc


LAST_RESULTS = None


def kernel(x, h_prev, W_z, U_z, b_z, W_a, U_a, v_a, W_h, U_h, b_h):
    global LAST_RESULTS
    use_bias = bool(np.any(np.asarray(b_z)) or np.any(np.asarray(b_h)))
    key = ("nc", use_bias)
    if key not in _CACHE:
        _CACHE[key] = build_nc(use_bias)
    nc = _CACHE[key]

    bf = ml_dtypes.bfloat16
    xbf = np.asarray(x, dtype=np.float32).astype(bf)
    hbf = np.asarray(h_prev, dtype=np.float32).astype(bf)
    wmats = [W_z, U_z, W_a, U_a, W_h, U_h]
    common = {
        "wall": np.ascontiguousarray(
            np.concatenate(
                [np.asarray(m, dtype=np.float32).T for m in wmats], axis=1
            ).astype(bf)
        )
    }
    common["va"] = np.ascontiguousarray(np.asarray(v_a, dtype=np.float32))
    # biases laid out [128 partition, 2 h-block]; z bias pre-scaled by 0.5
    common["bzh"] = np.ascontiguousarray(
        (0.5 * np.asarray(b_z, dtype=np.float32)).reshape(2, 128).T
    )
    common["bh"] = np.ascontiguousarray(
        np.asarray(b_h, dtype=np.float32).reshape(2, 128).T
    )

    in_maps = []
    for c in range(N_CORES):
        m = dict(common)
        m["xT"] = np.ascontiguousarray(xbf[c * B_CORE : (c + 1) * B_CORE].T)
        m["hT"] = np.ascontiguousarray(hbf[c * B_CORE : (c + 1) * B_CORE].T)
        in_maps.append(m)

    LAST_RESULTS = run_bass_kernel_spmd(nc, in_maps, core_ids=list(range(N_CORES)))
    outs = LAST_RESULTS.results
    res = np.empty((B_TOTAL, D), np.float32)
    for c in range(N_CORES):
        res[c * B_CORE : (c + 1) * B_CORE] = outs[c]["out"].T
    return res


# revision 10
# speedup vs baseline: 1.0951x; 1.0951x over previous
"""ImprovedGRUCell Trainium2 kernel (8-core data-parallel over batch).

v5 layout strategy — fully transposed (feature-major) compute:
  - batch sharded 8 ways (8192 rows/core); 256x256 weights replicated.
  - Host pre-transposes x/h to [256, B] bf16 per core, so activations load
    feature-major directly: SBUF tiles [128 part = feature, free = batch].
    No on-device transposes.
  - All GEMMs in transposed orientation S^T[h, b] = W @ x^T: weight tile
    stationary (lhsT = W.T block [128k, 128h]), activation moving (N=512).
    24 matmuls of [128,128]x[128,512] per 512-column super-tile.
  - softmax over hidden (partition dim): va scale + exp are per-partition
    ACT ops.  Denominator: ones_col.T @ E -> [1,512] psum row; ACT-copies
    to SBUF; rank-1 matmul (ones_row x row) re-broadcasts to [128,512]
    psum; reciprocal_approx_fast gives rb.  Normalization folded into h
    before att = E * (h * rb), which feeds U_h directly.
  - sigmoid avoided: z = 0.5*tanh(s/2)+0.5; blend h_t = h + u*(htl-h)
    computed as two fused scalar_tensor_tensor ops:
    m = (tz+1)*(htl-h) on DVE, out = 0.5*m + h on gpsimd.
  - 5-deep software pipeline keyed so every PE operand is produced a full
    iteration before use (no PE stalls): iter i runs cand(i-3),
    bcast(i-2), attn(i), z(i), denom(i-1) on the PE; the softmax chain
    E -> denom -> dr -> bcast -> recip -> att crosses one iteration per
    hop.  x/h loads prefetch one iteration ahead; weights ship as one
    DMA.
  - Output written bf16 transposed [256, B]; host transposes/casts to f32.
"""

import sys

sys.path.insert(0, "/opt/trn_rl_repo")

import ml_dtypes
import numpy as np

import concourse.bass as bass
import concourse.mybir as mybir
from concourse import bacc, tile
from concourse.bass_utils import run_bass_kernel_spmd

B_TOTAL = 65536
N_CORES = 8
B_CORE = B_TOTAL // N_CORES  # 8192
D = 256
ST = 512  # batch columns per super-tile
N_ST = B_CORE // ST  # 16
NW = 6  # weight matrices

F32 = mybir.dt.float32
BF16 = mybir.dt.bfloat16
AF = mybir.ActivationFunctionType
ALU = mybir.AluOpType

_CACHE = {}

# order inside the packed weight tensor
WNAMES = ("wzt", "uzt", "wat", "uat", "wht", "uht")
WIDX = {n: i for i, n in enumerate(WNAMES)}


def build_nc(use_bias=False):
    nc = bacc.Bacc(
        "TRN2",
        target_bir_lowering=False,
        debug=False,
        enable_asserts=False,
        num_devices=N_CORES,
    )

    x_d = nc.dram_tensor("xT", [D, B_CORE], BF16, kind="ExternalInput")
    h_d = nc.dram_tensor("hT", [D, B_CORE], BF16, kind="ExternalInput")
    w_d = nc.dram_tensor("wall", [D, NW * D], BF16, kind="ExternalInput")
    va_d = nc.dram_tensor("va", [D], F32, kind="ExternalInput")
    bz_d = nc.dram_tensor("bzh", [128, 2], F32, kind="ExternalInput")
    bh_d = nc.dram_tensor("bh", [128, 2], F32, kind="ExternalInput")
    out_d = nc.dram_tensor("out", [D, B_CORE], BF16, kind="ExternalOutput")

    with tile.TileContext(nc) as tc:
        with (
            tc.tile_pool(name="wpool", bufs=1) as wp,
            tc.tile_pool(name="io", bufs=4) as io,
            tc.tile_pool(name="wk", bufs=2) as wk,
            tc.tile_pool(name="psz", bufs=1, space="PSUM") as psZ,
            tc.tile_pool(name="psa", bufs=1, space="PSUM") as psA,
            tc.tile_pool(name="psc", bufs=1, space="PSUM") as psC,
            tc.tile_pool(name="psd", bufs=1, space="PSUM") as psD,
            tc.tile_pool(name="psr", bufs=1, space="PSUM") as psR,
        ):
            # ---- persistent weights (single DMA) ------------------------
            # wt[p, kb*(NW*D) + n*D + h] = Wn.T[kb*128 + p, h]
            wt = wp.tile([128, 2 * NW * D], BF16, tag="wt")
            nc.sync.dma_start(
                out=wt.rearrange("p (kb c) -> p kb c", kb=2),
                in_=w_d.ap().rearrange("(kb p) c -> p kb c", p=128),
            )

            def wsl(n, kb, hb):
                e = kb * (NW * D) + WIDX[n] * D + hb * 128
                return wt[:, e : e + 128]

            va_sb = wp.tile([128, 2], F32, tag="va")
            nc.sync.dma_start(
                out=va_sb[:], in_=va_d.ap().rearrange("(t p) -> p t", p=128)
            )
            ones_c = wp.tile([128, 1], BF16, tag="ones_c")
            nc.vector.memset(ones_c[:], 1.0)
            ones_r = wp.tile([1, 128], BF16, tag="ones_r")
            nc.vector.memset(ones_r[:], 1.0)
            if use_bias:
                bz_sb = wp.tile([128, 2], F32, tag="bz")
                nc.sync.dma_start(out=bz_sb[:], in_=bz_d.ap())
                bh_sb = wp.tile([128, 2], F32, tag="bh")
                nc.sync.dma_start(out=bh_sb[:], in_=bh_d.ap())

            # cross-iteration tile refs for the software pipeline
            xts, hts, tzs, Es, drs, atts, htls, dds = (
                {}, {}, {}, {}, {}, {}, {}, {},
            )

            def loads(st):
                b0 = st * ST
                xt = io.tile([128, 2 * ST], BF16, tag="xt", bufs=5)
                nc.sync.dma_start(
                    out=xt.rearrange("p (kb b) -> p kb b", kb=2),
                    in_=x_d.ap()[:, b0 : b0 + ST].rearrange(
                        "(kb p) b -> p kb b", p=128
                    ),
                )
                ht = io.tile([128, 2 * ST], BF16, tag="ht", bufs=7)
                nc.sync.dma_start(
                    out=ht.rearrange("p (kb b) -> p kb b", kb=2),
                    in_=h_d.ap()[:, b0 : b0 + ST].rearrange(
                        "(kb p) b -> p kb b", p=128
                    ),
                )
                xts[st], hts[st] = xt, ht

            def mm_pair(ps, wx, wh, rx, rh):
                """8 matmuls: ps[:, hb*ST:+ST] += Wx@rx + Wh@rh (2 k-blocks)."""
                for hb in range(2):
                    o = ps[:, hb * ST : (hb + 1) * ST]
                    for kb in range(2):
                        nc.tensor.matmul(
                            o,
                            wsl(wx, kb, hb),
                            rx[:, kb * ST : (kb + 1) * ST],
                            start=(kb == 0),
                            stop=False,
                        )
                    for kb in range(2):
                        nc.tensor.matmul(
                            o,
                            wsl(wh, kb, hb),
                            rh[:, kb * ST : (kb + 1) * ST],
                            start=False,
                            stop=(kb == 1),
                        )

            for it in range(N_ST + 5):
                # ==== input prefetch (one iteration ahead) ==============
                if it == 0:
                    loads(0)
                if it + 1 < N_ST:
                    loads(it + 1)

                # ==== stage E (it-4): blend + store =====================
                et = it - 4
                if 0 <= et < N_ST:
                    htb = hts.pop(et)
                    tzb, ddb = tzs.pop(et), dds.pop(et)
                    uu = wk.tile([128, 2 * ST], BF16, tag="uu")
                    nc.vector.tensor_scalar(
                        uu[:], tzb[:], 0.5, 0.5, op0=ALU.mult, op1=ALU.add
                    )
                    mm_ = wk.tile([128, 2 * ST], BF16, tag="mm_")
                    nc.vector.tensor_mul(mm_[:], uu[:], ddb[:])
                    ot = io.tile([128, 2 * ST], BF16, tag="ot", bufs=3)
                    nc.gpsimd.tensor_add(ot[:], mm_[:], htb[:])
                    nc.sync.dma_start(
                        out=out_d.ap()[:, et * ST : (et + 1) * ST].rearrange(
                            "(hb p) b -> p hb b", p=128
                        ),
                        in_=ot.rearrange("p (hb b) -> p hb b", hb=2),
                    )

                # ==== stage D (it-3): candidate branch + tanh + sub =====
                jt = it - 3
                if 0 <= jt < N_ST:
                    xtj = xts.pop(jt)
                    attj = atts.pop(jt)
                    pc = psC.tile([128, 2 * ST], F32, tag="pc")
                    mm_pair(pc, "wht", "uht", xtj, attj)
                    htl = wk.tile([128, 2 * ST], BF16, tag="htl", bufs=3)
                    if use_bias:
                        for hb in range(2):
                            sl = slice(hb * ST, (hb + 1) * ST)
                            nc.scalar.activation(
                                htl[:, sl], pc[:, sl], AF.Tanh,
                                bias=bh_sb[:, hb : hb + 1],
                            )
                    else:
                        nc.scalar.activation(htl[:], pc[:], AF.Tanh)
                    htls[jt] = htl
                    dd = wk.tile([128, 2 * ST], BF16, tag="dd", bufs=3)
                    nc.gpsimd.tensor_sub(dd[:], htl[:], hts[jt][:])
                    dds[jt] = dd

                # ==== stage C (it-2): denom bcast + normalize ===========
                kt = it - 2
                if 0 <= kt < N_ST:
                    rbp = psR.tile([128, ST], F32, tag="rbp")
                    nc.tensor.matmul(
                        rbp[:], ones_r[:], drs.pop(kt)[:], start=True, stop=True
                    )
                    rb = wk.tile([128, ST], F32, tag="rb")
                    nc.vector.reciprocal_approx_fast(out=rb[:], in_=rbp[:])
                    hr = wk.tile([128, 2 * ST], BF16, tag="hr")
                    for kb in range(2):
                        sl = slice(kb * ST, (kb + 1) * ST)
                        nc.vector.tensor_mul(hr[:, sl], hts[kt][:, sl], rb[:])
                    att = wk.tile([128, 2 * ST], BF16, tag="att", bufs=3)
                    nc.vector.tensor_mul(att[:], Es[kt][:], hr[:])
                    atts[kt] = att

                # ==== stage A: attention + z branches for tile `it` =====
                if it < N_ST:
                    xt, ht = xts[it], hts[it]
                    pa = psA.tile([128, 2 * ST], F32, tag="pa")
                    mm_pair(pa, "wat", "uat", xt, ht)
                    A = wk.tile([128, 2 * ST], BF16, tag="A")
                    nc.scalar.activation(A[:], pa[:], AF.Tanh)
                    E = wk.tile([128, 2 * ST], BF16, tag="E", bufs=4)
                    for hb in range(2):
                        sl = slice(hb * ST, (hb + 1) * ST)
                        nc.scalar.activation(
                            E[:, sl], A[:, sl], AF.Exp,
                            scale=va_sb[:, hb : hb + 1],
                        )
                    Es[it] = E

                    pz = psZ.tile([128, 2 * ST], F32, tag="pz")
                    mm_pair(pz, "wzt", "uzt", xt, ht)

                # ==== stage B (it-1): denominator row ===================
                dt_ = it - 1
                if 0 <= dt_ < N_ST:
                    Ed = Es[dt_]
                    pd = psD.tile([1, ST], F32, tag="pd")
                    for hb in range(2):
                        nc.tensor.matmul(
                            pd[:],
                            ones_c[:],
                            Ed[:, hb * ST : (hb + 1) * ST],
                            start=(hb == 0),
                            stop=(hb == 1),
                        )
                    dr = wk.tile([1, ST], BF16, tag="dr", bufs=2)
                    nc.scalar.activation(dr[:], pd[:], AF.Copy)
                    drs[dt_] = dr
                if 0 <= it - 3 < N_ST:
                    Es.pop(it - 3)

                # z-gate tanh (late: consumed 4 iterations later)
                if it < N_ST:
                    tz = wk.tile([128, 2 * ST], BF16, tag="tz", bufs=6)
                    if use_bias:
                        for hb in range(2):
                            sl = slice(hb * ST, (hb + 1) * ST)
                            nc.scalar.activation(
                                tz[:, sl], pz[:, sl], AF.Tanh,
                                bias=bz_sb[:, hb : hb + 1], scale=0.5,
                            )
                    else:
                        nc.scalar.activation(tz[:], pz[:], AF.Tanh, scale=0.5)
                    tzs[it] = tz

    nc.compile()
    return nc


LAST_RESULTS = None


def kernel(x, h_prev, W_z, U_z, b_z, W_a, U_a, v_a, W_h, U_h, b_h):
    global LAST_RESULTS
    use_bias = bool(np.any(np.asarray(b_z)) or np.any(np.asarray(b_h)))
    key = ("nc", use_bias)
    if key not in _CACHE:
        _CACHE[key] = build_nc(use_bias)
    nc = _CACHE[key]

    bf = ml_dtypes.bfloat16
    xbf = np.asarray(x, dtype=np.float32).astype(bf)
    hbf = np.asarray(h_prev, dtype=np.float32).astype(bf)
    wmats = [W_z, U_z, W_a, U_a, W_h, U_h]
    common = {
        "wall": np.ascontiguousarray(
            np.concatenate(
                [np.asarray(m, dtype=np.float32).T for m in wmats], axis=1
            ).astype(bf)
        )
    }
    common["va"] = np.ascontiguousarray(np.asarray(v_a, dtype=np.float32))
    # biases laid out [128 partition, 2 h-block]; z bias pre-scaled by 0.5
    common["bzh"] = np.ascontiguousarray(
        (0.5 * np.asarray(b_z, dtype=np.float32)).reshape(2, 128).T
    )
    common["bh"] = np.ascontiguousarray(
        np.asarray(b_h, dtype=np.float32).reshape(2, 128).T
    )

    in_maps = []
    for c in range(N_CORES):
        m = dict(common)
        m["xT"] = np.ascontiguousarray(xbf[c * B_CORE : (c + 1) * B_CORE].T)
        m["hT"] = np.ascontiguousarray(hbf[c * B_CORE : (c + 1) * B_CORE].T)
        in_maps.append(m)

    LAST_RESULTS = run_bass_kernel_spmd(nc, in_maps, core_ids=list(range(N_CORES)))
    outs = LAST_RESULTS.results
    res = np.empty((B_TOTAL, D), np.float32)
    for c in range(N_CORES):
        res[c * B_CORE : (c + 1) * B_CORE] = outs[c]["out"].T
    return res
